# revision 38
# baseline (speedup 1.0000x reference)
"""Trainium2 Bass kernel for nn_AttnFuser (fused MHA + FFN transformer block).

Sharding: 8 cores = 2 batches x 4 query-token slices of 512. Each core computes
the full block for its 512 query tokens; K/V projection over the full context
of its batch is replicated within each 4-core batch group (no collectives).

On-chip layout is feature-major ([feature, token]) for Q/K and the FFN, and
token-major for V. The Q/K/V projections and the attention AV matmul run in
fp8e4/fp8e5 with DoubleRow perf mode (2 contraction rows per PE cell -> half
the matmul cycles); operands are pre-scaled (weights x512, activations x16,
V x32) so fp8's narrow mantissa lands on well-conditioned data, and the
scales are folded into the PSUM->SBUF copies / softmax normalization. The
QK matmul and the FFN stay bf16 (fp8 there costs too much accuracy).

The softmax exp over the [2048 x 1024] score block of each head pair is the
single largest elementwise job, so it is split across two engines: ACT
computes true exp into fp8e5, and DVE computes a Schraudolph-style exp for
the other chunks -- y = round(score * 0.5*log2(e) + 62) written as uint8 IS
the fp8e5 bit pattern of ~exp(score/8) (the +62 bias error is a uniform
factor that cancels in softmax; fp32->uint8 saturation maps the negative
tail to +0.0). Softmax denominators come free from a ones-column in V, are
reciprocal'd per head pair, and each pair's normalization + FFN-LN stats
are pipelined into the next pair's attention so nothing serializes at the
attention->FFN boundary.

SBUF is tight, so large tiles share pool tags in strict temporal chains
(e.g. the context tile's slot is later reused by the FFN hidden activations).
"""
import os
import numpy as np
import ml_dtypes

BF16 = ml_dtypes.bfloat16
E4 = ml_dtypes.float8_e4m3
E5 = ml_dtypes.float8_e5m2
XS = 16.0      # fp8 activation pre-scale
WS = 512.0     # fp8 weight pre-scale
VS = 32.0      # V value scale (cancels in softmax normalization)
PSC = 1.0 / (XS * WS)   # PSUM descale for fp8 projections
SCH_A = 0.125 * 4.0 / float(np.log(2.0))   # Schraudolph slope (folds the /8)
SCH_B = 62.0                               # Schraudolph bias (cancels in softmax)


def _maybe_patch_ldw_opt():
    """KERNEL_LDW_OPT=1: flip walrus --enable-ldw-opt to true (dedups/pipelines
    LDWEIGHTS). Verified against the reference on every run."""
    if os.environ.get("KERNEL_LDW_OPT") != "1":
        return
    import concourse.bass_utils as bu
    if getattr(bu, "_ldw_patched", False):
        return
    orig = bu.run_command

    def run_command_ldw(argv, **kw):
        argv = ["--enable-ldw-opt=true" if a == "--enable-ldw-opt=false" else a
                for a in argv]
        return orig(argv, **kw)

    bu.run_command = run_command_ldw
    bu._ldw_patched = True

D, T, M, H, DH, DFF = 1024, 512, 2048, 16, 64, 4096
NCH = D // 128      # 8 feature chunks
TTK = M // 512      # 4 context token tiles
MC = M // 128       # 16 context chunks
B, N = 2, 2048      # full problem dims

_BUILT = {}


def _patch_tile_drain():
    """This walrus build rejects >1 sem wait on an InstDrain (TPB_CTRL
    setupSyncWait). Split the TileContext tail-drain waits onto nop insts."""
    import concourse.tile as tile_mod
    from concourse import mybir
    from concourse.vector_clock import ScopedClock
    if getattr(tile_mod.TileContext, "_drain_patched", False):
        return

    def _drain_and_barrier(self, tick_clock, wait_clock):
        nc = self.nc
        drain_inst = nc.sync.drain()
        wait_clock.add_sem_waits(
            drain_inst.ins, ScopedClock({None: tick_clock.global_clock}))
        si = drain_inst.ins.sync_info
        waits = list(si.on_wait or []) if si else []
        if len(waits) > 1:
            drain_inst.ins.sync_info = mybir.SyncInfo(
                on_wait=waits[:1], on_update=list(si.on_update or []))
            for w in waits[1:]:
                nop = nc.sync.nop(nofuse=True, hint="split_drain_wait")
                nop.ins.sync_info = mybir.SyncInfo(on_wait=[w], on_update=[])
        nc.all_engine_barrier()
        popped = nc._tile_sem_poison_stack.pop()
        assert popped is self._sem_poison
        nc.clear_and_free_semaphores(list(self.sems.allocated().values()))
        nc.all_engine_barrier()

    tile_mod.TileContext._drain_and_barrier = _drain_and_barrier
    tile_mod.TileContext._drain_patched = True


def _split_sync_waits(nc, max_waits=1):
    """This walrus build rejects instructions carrying more than ~1 sem wait
    (setupSyncWait: 'Too many sync wait commands'). Hoist extra waits onto
    same-engine NOPs inserted immediately before the instruction — the engine
    executes them in order, so all waits are still satisfied before the op."""
    from concourse import mybir
    n = 0
    for f in nc.m.functions:
        for bb in f.blocks:
            insts = bb.instructions
            new = []
            for inst in insts:
                si = getattr(inst, "sync_info", None)
                waits = list(si.on_wait) if si and si.on_wait else []
                if len(waits) > max_waits:
                    for w in waits[max_waits:]:
                        nop = mybir.InstNoOp(
                            name=f"wsplit_{n}",
                            sync_info=mybir.SyncInfo(on_wait=[w], on_update=[]),
                            bass_nofuse=True,
                            engine=inst.engine,
                        )
                        nc.register_instruction(nop)
                        n += 1
                        new.append(nop)
                    inst.sync_info = mybir.SyncInfo(
                        on_wait=waits[:max_waits],
                        on_update=list(si.on_update or []))
                new.append(inst)
            insts[:] = new
    return n


def input_specs(with_tq, with_tk, skip_fn=False, skip_b2=False):
    """(name, shape, np_dtype) for every DRAM input."""
    sp = [
        ("xq", (D, T), E4), ("xqf", (D, T), np.float32),
        ("xc", (D, M), E4),
        ("wq", (NCH, 128, NCH, 128), E4), ("wk", (NCH, 128, NCH, 128), E4),
        ("wv", (128, NCH, D), E4),
        ("w1", (NCH, 128, NCH, 512), E4),
        ("w2", (NCH, 2, 128, DFF // 256, 128), BF16),
        ("bq", (D,), np.float32), ("bk", (D,), np.float32),
        ("b1", (DFF,), np.float32), ("b2", (D,), np.float32),
        ("fng", (D,), np.float32), ("fnb", (D,), np.float32),
        ("cq2", (128, T), BF16), ("sq2", (128, T), BF16),
        ("ck2", (128, M), BF16), ("sk2", (128, M), BF16),
        ("bo16", (128, 8 * 16), BF16),
        ("indall", (16, D), BF16),
        ("ind2", (2, 128), BF16),
        ("perm", (128, 128), BF16),
        ("ones128f", (128, 1), np.float32),  # value 1/1024 (FFN stats lhsT)
        ("ones128b", (128, 1), BF16),        # bf16 twin for bf16 stats matmuls
        ("xqfm", (1, T), np.float32),        # feature-mean of xqf per token
        ("onesr", (1, 128), BF16),           # bf16 ones row (FFN bc lhsT)
    ]
    if with_tq:
        sp.append(("tq", (128, T), BF16))
    if with_tk:
        sp.append(("tk", (128, M), BF16))
    return sp


def build(with_tq=False, with_tk=False, skip_fn=False, skip_b2=False):
    import concourse.bass as bass
    import concourse.mybir as mybir
    import concourse.tile as tile
    from contextlib import ExitStack

    _patch_tile_drain()
    AF = mybir.ActivationFunctionType
    OP = mybir.AluOpType
    DR = mybir.MatmulPerfMode.DoubleRow
    bf = mybir.dt.bfloat16
    f32 = mybir.dt.float32
    f8 = mybir.dt.float8e4
    f8e5 = mybir.dt.float8e5
    u8 = mybir.dt.uint8

    nc = bass.Bass()
    dram = {}
    for name, shape, npdt in input_specs(with_tq, with_tk, skip_fn, skip_b2):
        dt = {BF16: bf, E4: f8, E5: f8e5}.get(npdt, f32)
        dram[name] = nc.declare_dram_parameter(name, list(shape), dt, isOutput=False)
    out_d = nc.declare_dram_parameter("out", [D, T], f32, isOutput=True)

    with tile.TileContext(nc) as tc, ExitStack() as ctx:
        const = ctx.enter_context(tc.tile_pool(name="const", bufs=1))
        u = ctx.enter_context(tc.tile_pool(name="u", bufs=1))
        wt = ctx.enter_context(tc.tile_pool(name="wt", bufs=2))
        # PSUM: one 3-deep ring of 2-bank tiles (scores/proj/FFN/rope; rope
        # tiles are quick-drain) + a pinned 2-bank tile for the two AV
        # accumulators = 8 banks total
        p2p = ctx.enter_context(tc.tile_pool(name="p2p", bufs=3, space="PSUM"))

        def p2(name):
            return p2p.tile([128, 1024], f32, tag="p2", name=name)

        def pav(name):
            return p2p.tile([128, 1024], f32, tag="pav", bufs=1, name=name)

        def pr(name):
            return p2(name)

        def load(pool, name, rearr=None, tag=None, eng=None, **kw):
            src = dram[name][:]
            if rearr is not None:
                src = src.rearrange(rearr, **kw)
            t = pool.tile(list(src.shape), src.dtype, tag=tag or name, name=name)
            (eng or nc.gpsimd).dma_start(out=t[:], in_=src)
            return t

        # -------- DMA front-load: what the first ~40us needs, in need order.
        # sync queue: xq then the wq stream (issued inside proj); gpsimd+scalar
        # split the context, then wv, then the small constants.
        xc_sb = u.tile([128, NCH, M], f8, tag="cA", name="xc")       # cA: xc->h1
        xq_sb = u.tile([128, NCH, T], f8, tag="cE", name="xqbf")     # cE: xq->h
        nc.sync.dma_start(out=xq_sb[:], in_=dram["xq"][:].rearrange("(c p) n -> p c n", p=128))
        bq_sb = load(const, "bq", "(c p) -> p c", p=128)
        bk_sb = load(const, "bk", "(c p) -> p c", p=128, eng=nc.scalar)
        xcr = dram["xc"][:].rearrange("(c p) m -> p c m", p=128)
        engs = [nc.gpsimd, nc.scalar]
        for c in range(NCH):
            engs[c % 2].dma_start(out=xc_sb[:, c, :], in_=xcr[:, c, :])
        wv_sb = u.tile([128, NCH, D], f8, tag="cD", name="wvsb")     # cD: wv->OT
        nc.gpsimd.dma_start(out=wv_sb[:, 0:4, :], in_=dram["wv"][:][:, 0:4, :])
        nc.scalar.dma_start(out=wv_sb[:, 4:8, :], in_=dram["wv"][:][:, 4:8, :])

        bo16 = load(const, "bo16")
        indall = load(const, "indall", eng=nc.scalar)
        ind2 = load(const, "ind2")
        perm = load(const, "perm", eng=nc.scalar)
        ones128f = load(const, "ones128f")
        ones128b = load(const, "ones128b")
        xqfm_sb = load(const, "xqfm")
        onesr = load(const, "onesr", eng=nc.scalar)
        eps = const.tile([128, 1], f32, tag="eps", name="eps")
        nc.vector.memset(eps[:], 1e-5)
        # af bias = ln(XS): folds the fp8 activation pre-scale for the FFN
        # input into the LN rstd (h_sb is stored fp8e4 scaled by XS)
        lnxs = const.tile([1, 1], f32, tag="lnxs", name="lnxs")
        nc.vector.memset(lnxs[:], float(np.log(XS)))
        salt = int(os.environ.get("KERNEL_SALT", "0"))
        if salt:
            # cache-busting dummy (changes BIR bytes so the NEFF cache misses)
            dummy = const.tile([1, 1], f32, tag="dummy", name="dummy")
            nc.vector.memset(dummy[:], float(salt))

        # big activations (tags are temporal chains -- comments show the chain)
        # K is per-chunk so each o_c can take over chunk c's slot right after
        # pair c's QK matmuls (a single KT tile would hold its slot until
        # pair 7 and deadlock the per-pair normalization pipeline)
        KTs = [u.tile([128, M], bf, tag=f"cKT{c}", name=f"KT{c}")
               for c in range(NCH)]
        QT = u.tile([128, NCH, T], bf, tag="cQT", name="QT")
        V = u.tile([128, MC, H, DH + 1], f8, tag="cC", name="V")     # cC: V->x2f
        aK = u.tile([16, M], bf, tag="caK", name="aK")               # caK: aK->A2
        aQ = u.tile([16, T], bf, tag="caQ", name="aQ")               # caQ: aQ->af
        xqf_sb = u.tile([128, NCH, T], f32, tag="cF", name="xqf")

        # V ones-columns (col DH for every head); value VS cancels against the
        # VS scale of O in the softmax normalization
        nc.vector.memset(V[:, :, :, DH:DH + 1], VS)

        # ---------------- projections (fp8 DoubleRow) ----------------
        def proj_featmajor(wname, x_sb, ntt, outs, bias):
            wr = dram[wname][:]
            npairs = (ntt + 1) // 2
            for m in range(NCH):
                wtile = wt.tile([128, NCH, 128], f8, tag="wqk", name=f"w_{wname}_{m}")
                nc.sync.dma_start(out=wtile[:], in_=wr[m])
                pts = [p2(f"ps_{wname}_{m}_{j}") for j in range(npairs)]
                halves = [pts[n // 2][:, (n % 2) * 512:(n % 2) * 512 + 512]
                          for n in range(ntt)]
                for kc in range(NCH // 2):
                    for n in range(ntt):
                        nc.tensor.matmul(halves[n], lhsT=wtile[:, 2 * kc:2 * kc + 2, :],
                                         rhs=x_sb[:, 2 * kc:2 * kc + 2, n * 512:(n + 1) * 512],
                                         start=(kc == 0), stop=(kc == NCH // 2 - 1),
                                         perf_mode=DR, skip_group_check=True)
                for j in range(npairs):
                    wid = min(1024, (ntt - 2 * j) * 512)
                    nc.scalar.activation(out=outs[m][:, 1024 * j:1024 * j + wid],
                                         in_=pts[j][:, 0:wid], func=AF.Identity,
                                         bias=bias[:, m:m + 1], scale=PSC)

        # ---------------- per-head QK layernorm + rope ----------------
        # The per-head mean is projected out of Wq/Wk host-side, so LN reduces
        # to a pure rstd scale: X = (C2*x + S2*rot(x)) * A [+ Tadd].
        def ln_sq_stats(Xs, ntt, a_sb, pref):
            # sq PSUM rides the pot ring (idle until attention), so the stats
            # never contend with projection/score tiles in the p2 ring
            for tt in range(ntt):
                ts_ = slice(tt * 512, (tt + 1) * 512)
                sqp = p2(f"sq_{pref}{tt}")
                sq = sqp[0:16, 0:512]
                for c in range(NCH):
                    xs = Xs[c][:, ts_]
                    x2 = u.tile([128, 512], bf, tag="x2", bufs=2, name=f"x2_{pref}{tt}_{c}")
                    nc.vector.tensor_mul(out=x2[:], in0=xs, in1=xs)
                    # one-hot block lhsT accumulates chunk c's two head rows
                    nc.tensor.matmul(sq, lhsT=bo16[:, c * 16:(c + 1) * 16], rhs=x2[:],
                                     start=(c == 0), stop=(c == NCH - 1),
                                     skip_group_check=True)
                # rstd = exp(-0.5*ln(E[x^2]+eps)): two ACT table ops straight
                # from PSUM (inputs are centered, so E[x^2] is the variance)
                lnv = u.tile([16, 512], f32, tag="csd", name=f"lnv_{pref}{tt}")
                nc.scalar.activation(out=lnv[:], in_=sq, func=AF.Ln,
                                     bias=eps[0:16, :], scale=1.0)
                with nc.allow_low_precision("bf16 rstd for broadcast matmul"):
                    nc.scalar.activation(out=a_sb[:, ts_], in_=lnv[:], func=AF.Exp,
                                         scale=-0.5)

        def ln_rope(Xs, ntt, a_sb, ctab, stab, ttab, pref, only_c=None):
            # per-512-token slices through the 1-bank prp ring; the aps
            # broadcast is emitted right before its single consumer to keep
            # PSUM dwell minimal
            for c in (range(NCH) if only_c is None else [only_c]):
                for tt in range(ntt):
                    ts_ = slice(tt * 512, (tt + 1) * 512)
                    rot = pr(f"rot_{pref}{c}_{tt}")
                    nc.tensor.matmul(rot[:, 0:512], lhsT=perm[:], rhs=Xs[c][:, ts_],
                                     start=True, stop=True, skip_group_check=True)
                    zz1 = u.tile([128, 512], bf, tag="czz1", bufs=2,
                                 name=f"zz1_{pref}{c}_{tt}")
                    zz2 = u.tile([128, 512], bf, tag="czz2", bufs=2,
                                 name=f"zz2_{pref}{c}_{tt}")
                    nc.vector.tensor_mul(out=zz2[:], in0=rot[:, 0:512],
                                         in1=stab[:, ts_])
                    nc.vector.tensor_mul(out=zz1[:], in0=Xs[c][:, ts_],
                                         in1=ctab[:, ts_])
                    nc.vector.tensor_add(out=zz1[:], in0=zz1[:], in1=zz2[:])
                    aps = pr(f"aps_{pref}{c}_{tt}")
                    nc.tensor.matmul(aps[:, 0:512], lhsT=indall[:, c * 128:(c + 1) * 128],
                                     rhs=a_sb[:, ts_], start=True, stop=True,
                                     skip_group_check=True)
                    if ttab is None:
                        nc.vector.tensor_mul(out=Xs[c][:, ts_], in0=zz1[:],
                                             in1=aps[:, 0:512])
                    else:
                        nc.vector.tensor_mul(out=zz1[:], in0=zz1[:], in1=aps[:, 0:512])
                        nc.vector.tensor_add(out=Xs[c][:, ts_], in0=zz1[:],
                                             in1=ttab[:, ts_])

        QTs = [QT[:, c, :] for c in range(NCH)]
        proj_featmajor("wq", xq_sb, 1, QTs, bq_sb)
        cq2 = load(u, "cq2", eng=nc.sync); sq2 = load(u, "sq2", eng=nc.sync)
        tqt = load(const, "tq") if with_tq else None
        tkt = load(const, "tk") if with_tk else None
        # K projection next: a dense PE burst that keeps the ramped clock
        # while DVE chews the Q stats + rope emitted right after
        proj_featmajor("wk", xc_sb, TTK, [t[:] for t in KTs], bk_sb)
        ck2 = load(u, "ck2", eng=nc.sync); sk2 = load(u, "sk2", eng=nc.sync)
        ln_sq_stats(QTs, 1, aQ, "q")
        ln_rope(QTs, 1, aQ, cq2, sq2, tqt, "q")

        def v_proj_pair(g):
            # two context chunks of the V projection (PE work that overlaps
            # the DVE-bound K layernorm+rope); fp8 DoubleRow over kc pairs
            for mc in (2 * g, 2 * g + 1):
                ps = p2(f"ps_v_{mc}")
                for kc in range(NCH // 2):
                    lh = xc_sb[:, 2 * kc:2 * kc + 2, mc * 128:(mc + 1) * 128]
                    nc.tensor.matmul(ps[:, 0:512], lhsT=lh,
                                     rhs=wv_sb[:, 2 * kc:2 * kc + 2, 0:512],
                                     start=(kc == 0), stop=(kc == NCH // 2 - 1),
                                     perf_mode=DR, skip_group_check=True)
                    nc.tensor.matmul(ps[:, 512:1024], lhsT=lh,
                                     rhs=wv_sb[:, 2 * kc:2 * kc + 2, 512:1024],
                                     start=(kc == 0), stop=(kc == NCH // 2 - 1),
                                     perf_mode=DR, skip_group_check=True)
                pv = ps[:].rearrange("p (hh d) -> p hh d", d=DH)
                nc.scalar.activation(out=V[:, mc, 0:H, 0:DH], in_=pv[:],
                                     func=AF.Copy, scale=PSC * VS)

        # xqf (residual) loads late (first use is pair-0 normalization, inside
        # pair 2 of the attention); OT takes over wv's slot
        nc.sync.dma_start(out=xqf_sb[:], in_=dram["xqf"][:].rearrange("(c p) n -> p c n", p=128))
        OT = u.tile([128, NCH, T], bf, tag="cD", name="OT")

        # V projection runs upfront (a dense PE burst that overlaps the
        # DVE-bound Q-rope tail); attention pair 0's AV needs all of V
        for g in range(NCH):
            v_proj_pair(g)
        ln_sq_stats(KTs, TTK, aK, "k")
        # w1 group 0/1 prefetch into dedicated slots: the DMA queue is idle
        # here and the shared att-slot copies couldn't start until the last
        # attention wave drained
        w1r = dram["w1"][:]
        w1g01 = u.tile([128, 2, NCH, 512], f8, tag="cw1p", name="w1g01")
        nc.sync.dma_start(out=w1g01[:, 0], in_=w1r[0])
        nc.sync.dma_start(out=w1g01[:, 1], in_=w1r[1])

        # ---------------- fused K-rope + attention ----------------
        # Attention pair p only needs rope'd K chunk p, so pair p-1 rides
        # right behind rope chunk p: the PE stream alternates [rope-PE(c),
        # QK/AV(c-1)] in long runs (keeps the HAM clock-gate at 8/8), while
        # DVE/ACT run rope elementwise + the exp split concurrently.
        # Head pairs: the two K=64 QK matmuls run as concurrent PE row-tiles
        # (rows 0-63 / 64-127) into the two banks of one PSUM tile. The exp of
        # each [128,1024] score chunk splits ACT (true exp) / DVE (Schraudolph
        # uint8 -> fp8e5 bits). Pair c's normalization, residual add and
        # FFN-LN stat accumulation run inside pair c+1's instruction stream so
        # the reciprocal latency hides under attention matmuls.
        WV = 4                      # ctx chunks per wave
        os_ = [None] * NCH
        # x2f rides in the context slot (cA, dead after the V projection) --
        # NOT in V's slot: V is read by every pair's AV matmul, and an aliased
        # write here would stall the in-order ACT queue into a cycle
        x2f = u.tile([128, NCH, T], bf, tag="cA", name="x2f")
        acc = u.tile([1, 1024], f32, tag="cacc", name="acc")   # [mean | meansq]
        dall2s = [None] * NCH

        def norm_pair(c):
            # softmax-normalize chunk c (heads 2c/2c+1), residual, FFN-LN stats
            os_[c] = u.tile([128, T], f32, tag=f"cKT{c}", name=f"o_{c}")
            xs2 = x2f[:, c, :]
            rt = u.tile([2, 512], f32, tag="crt", bufs=1, name=f"rt_{c}")
            nc.scalar.activation(out=rt[:], in_=dall2s[c][:], func=AF.Ln)
            rl = u.tile([2, 512], bf, tag="crl", bufs=1, name=f"rl_{c}")
            with nc.allow_low_precision("bf16 softmax reciprocal broadcast"):
                nc.scalar.activation(out=rl[:], in_=rt[:], func=AF.Exp, scale=-1.0)
            rps = pr(f"rps_{c}")
            nc.tensor.matmul(rps[:, 0:512], lhsT=ind2[:], rhs=rl[:],
                             start=True, stop=True)
            rsb = u.tile([128, 512], bf, tag="crsb", bufs=2, name=f"rsb_{c}")
            nc.scalar.activation(out=rsb[:], in_=rps[:, 0:512], func=AF.Copy)
            nc.vector.tensor_mul(out=OT[:, c, :], in0=OT[:, c, :], in1=rsb[:])
            nc.vector.tensor_add(out=os_[c][:], in0=xqf_sb[:, c, :],
                                 in1=OT[:, c, :])
            with nc.allow_low_precision("bf16 squares for FFN-LN variance"):
                nc.vector.tensor_mul(out=xs2, in0=os_[c][:], in1=os_[c][:])
            # stats in bf16 (fp32 matmuls run LOW_HIGH at 4x the cycles); the
            # residual's feature-mean is added from a host-computed row later
            smm = p2(f"smm_{c}")
            nc.tensor.matmul(smm[0:1, 0:512], lhsT=ones128b[:], rhs=OT[:, c, :],
                             start=True, stop=True, skip_group_check=True)
            nc.tensor.matmul(smm[0:1, 512:1024], lhsT=ones128b[:], rhs=xs2,
                             start=True, stop=True, skip_group_check=True)
            if c == 0:
                nc.vector.tensor_copy(out=acc[:], in_=smm[0:1, :])
            else:
                nc.vector.tensor_add(out=acc[:], in0=acc[:], in1=smm[0:1, :])

        def attn_pair(pair):
            he, ho = 2 * pair, 2 * pair + 1
            c = pair
            ot2 = pav(f"ot_{pair}")
            ote, oto = ot2[:, 0:512], ot2[:, 512:1024]
            for w in range(MC // WV):
                # waves 0-2: ACT true exp into fp8e4 (scores exp(s/8) stay
                # well under e4m3's 448 max, and e4m3 quantizes 4x finer than
                # e5m2); wave 3 carries the DVE Schraudolph chunks, whose
                # uint8 bit-trick is e5m2-only
                att = u.tile([128, WV, 1024], f8 if w < 3 else f8e5,
                             tag=("catt8a" if w % 2 == 0 else "catt8b"),
                             name=f"att_{pair}_{w}")
                for i in range(WV):
                    mc = w * WV + i
                    sp = p2(f"sp_{pair}_{mc}")
                    nc.tensor.matmul(sp[:, 0:512],
                                     lhsT=KTs[c][0:64, mc * 128:(mc + 1) * 128],
                                     rhs=QT[0:64, c, :], start=True, stop=True,
                                     tile_position=(0, 0))
                    nc.tensor.matmul(sp[:, 512:1024],
                                     lhsT=KTs[c][64:128, mc * 128:(mc + 1) * 128],
                                     rhs=QT[64:128, c, :], start=True, stop=True,
                                     tile_position=(64, 0))
                    if mc < 13:
                        with nc.allow_low_precision("fp8 softmax weights"):
                            nc.scalar.activation(out=att[:, i, :], in_=sp[:],
                                                 func=AF.Exp, scale=0.125)
                    else:
                        attu = att[:, i, :].bitcast(u8)
                        with nc.allow_low_precision("schraudolph exp to fp8e5 bits"):
                            nc.vector.tensor_scalar(out=attu, in0=sp[:],
                                                    scalar1=SCH_A, scalar2=SCH_B,
                                                    op0=OP.mult, op1=OP.add)
                for i2 in range(WV // 2):
                    mcp = w * WV + 2 * i2
                    nc.tensor.matmul(ote[0:DH + 1, :],
                                     lhsT=V[:, mcp:mcp + 2, he, :],
                                     rhs=att[:, 2 * i2:2 * i2 + 2, 0:512],
                                     start=(mcp == 0), stop=(mcp == MC - 2),
                                     perf_mode=DR, skip_group_check=True)
                    nc.tensor.matmul(oto[0:DH + 1, :],
                                     lhsT=V[:, mcp:mcp + 2, ho, :],
                                     rhs=att[:, 2 * i2:2 * i2 + 2, 512:1024],
                                     start=(mcp == 0), stop=(mcp == MC - 2),
                                     perf_mode=DR, skip_group_check=True)
                if w == 1 and pair >= 1:
                    # previous pair's softmax normalization rides here: its
                    # reciprocal input became ready ~2 waves ago, so the PE
                    # broadcast matmul below never stalls the attention
                    norm_pair(pair - 1)
            # stash unnormalized O and the denominators (row 64 = ones-column)
            dall2 = u.tile([2, 512], bf, tag="cdall", bufs=1, name=f"dall2_{pair}")
            dall2s[pair] = dall2
            # rows 0..63 = O, row 64 = denominator (ones-column): one copy
            # covers both; the denominator row DMAs out before the odd head's
            # stash overwrites partitions 64+
            nc.vector.tensor_copy(out=OT[0:65, c, :], in_=ote[0:65, :])
            nc.sync.dma_start(out=dall2[0:1, :], in_=OT[64:65, c, :])
            # odd head: O sits at PSUM rows 0..63 but belongs at partitions
            # 64..127 of OT; shift with a bounce through SBUF + gpsimd DMA
            tmp = u.tile([128, 512], bf, tag="cotmp", bufs=1, name=f"otmp_{ho}")
            nc.vector.tensor_copy(out=tmp[0:65, :], in_=oto[0:65, :])
            nc.gpsimd.dma_start(out=OT[64:128, c, :], in_=tmp[0:64, :])
            nc.gpsimd.dma_start(out=dall2[1:2, :], in_=tmp[64:65, :])

        # fused K-rope + attention: rope chunk c's PE/DVE work interleaves
        # with attention pair c-1, keeping the PE stream dense (p-state) while
        # DVE alternates rope and Schraudolph/normalization work
        ln_rope(KTs, TTK, aK, ck2, sk2, tkt, "k", only_c=0)
        for c in range(1, NCH):
            ln_rope(KTs, TTK, aK, ck2, sk2, tkt, "k", only_c=c)
            attn_pair(c - 1)
        attn_pair(NCH - 1)
        # keep the PE p-state up through the DVE-bound softmax/LN transition:
        # accumulate throwaway matmuls into the (now free) pav banks so the
        # clock doesn't halve right before the FFN
        dmy1 = pav("dmy1")
        for i in range(24):
            nc.tensor.matmul(dmy1[:, 0:512], lhsT=perm[:], rhs=sk2[:, 0:512],
                             start=(i == 0), stop=(i == 23), skip_group_check=True)
        norm_pair(NCH - 1)

        # ---------------- FFN ----------------
        b2_sb = load(const, "b2", "(c p) -> p c", p=128)
        b1_sb = load(const, "b1", "(c p) -> p c", p=128)
        fng_sb = load(const, "fng", "(c p) -> p c", p=128)
        fnb_sb = load(const, "fnb", "(c p) -> p c", p=128)
        mu0 = u.tile([1, 512], f32, tag="cmu", name="mu0")
        nc.vector.tensor_add(out=mu0[:], in0=acc[0:1, 0:512], in1=xqfm_sb[:])
        muf = mu0[:]
        t1f = u.tile([1, 512], f32, tag="ct1", name="t1f")
        nc.vector.tensor_mul(out=t1f[:], in0=muf, in1=muf)
        varf = u.tile([1, 512], f32, tag="cvar", name="varf")
        nc.vector.tensor_tensor(out=varf[:], in0=acc[0:1, 512:1024], in1=t1f[:],
                                op=OP.subtract)
        lnf = u.tile([1, 512], f32, tag="csd", name="lnf")
        nc.scalar.activation(out=lnf[:], in_=varf[:], func=AF.Ln, bias=eps[0:1, :],
                             scale=1.0)
        af = u.tile([1, 512], bf, tag="caQ", name="af")
        with nc.allow_low_precision("bf16 rstd for broadcast matmul"):
            # bias=ln(XS): af = XS * rstd, so h_sb comes out pre-scaled for fp8
            nc.scalar.activation(out=af[:], in_=lnf[:], func=AF.Exp, scale=-0.5,
                                 bias=lnxs[:])
        bff = u.tile([1, 512], bf, tag="cbQ", name="bff")
        nc.vector.tensor_mul(out=bff[:], in0=muf, in1=af[:])
        abf = p2("abf")
        nc.tensor.matmul(abf[:, 0:512], lhsT=onesr[:], rhs=af[:], start=True, stop=True)
        nc.tensor.matmul(abf[:, 512:1024], lhsT=onesr[:], rhs=bff[:], start=True, stop=True)
        A2 = u.tile([128, 512], bf, tag="caK", name="A2")
        nc.scalar.activation(out=A2[:], in_=abf[:, 0:512], func=AF.Copy)
        B2 = u.tile([128, 512], bf, tag="cbK", name="B2")
        nc.scalar.activation(out=B2[:], in_=abf[:, 512:1024], func=AF.Copy)
        # second p-state filler block: covers the DVE h_sb chain below
        dmy2 = pav("dmy2")
        for i in range(36):
            nc.tensor.matmul(dmy2[:, 0:512], lhsT=perm[:], rhs=sk2[:, 0:512],
                             start=(i == 0), stop=(i == 35), skip_group_check=True)

        h_sb = u.tile([128, NCH, T], f8, tag="cE", name="hsb")
        for c in range(NCH):
            with nc.allow_low_precision("fp8 FFN activations"):
                if skip_fn:
                    tn = u.tile([128, 512], f32, tag="ck2", name=f"tn_{c}")
                    nc.vector.tensor_mul(out=tn[:], in0=os_[c][:], in1=A2[:])
                    nc.vector.tensor_tensor(out=h_sb[:, c, :], in0=tn[:], in1=B2[:],
                                            op=OP.subtract)
                else:
                    tn = u.tile([128, 512], f32, tag="ck2", name=f"tn_{c}")
                    nc.vector.tensor_mul(out=tn[:], in0=os_[c][:], in1=A2[:])
                    nc.vector.tensor_tensor(out=tn[:], in0=tn[:], in1=B2[:], op=OP.subtract)
                    nc.vector.tensor_scalar(out=h_sb[:, c, :], in0=tn[:],
                                            scalar1=fng_sb[:, c:c + 1],
                                            scalar2=fnb_sb[:, c:c + 1],
                                            op0=OP.mult, op1=OP.add)

        # FFN matmul 1 (fp8 DoubleRow) + exact GELU (weights streamed as
        # 0.5MB group tiles through the attention att-tile slots); the
        # 1/(XS*WS) descale folds into gelu's input scale, so h1 is the true
        # gelu output, stored fp8e4 (range ~[-0.17, 6] fits e4m3 natively)
        h1_sb = u.tile([128, DFF // 128, T], bf, tag="cA", name="h1")
        for g in range(NCH):
            if g < 2:
                w1g = w1g01[:, g]
            else:
                w1g = u.tile([128, NCH, 512], f8,
                             tag=("catt8a" if g % 2 == 0 else "catt8b"),
                             name=f"w1g_{g}")
                nc.sync.dma_start(out=w1g[:], in_=w1r[g])
            for mm in range(4):
                m = 4 * g + mm
                ps = p2(f"ps_h1_{m}")
                for kc in range(NCH // 2):
                    nc.tensor.matmul(ps[:, 0:512],
                                     lhsT=w1g[:, 2 * kc:2 * kc + 2,
                                              mm * 128:(mm + 1) * 128],
                                     rhs=h_sb[:, 2 * kc:2 * kc + 2, :],
                                     start=(kc == 0), stop=(kc == NCH // 2 - 1),
                                     perf_mode=DR, skip_group_check=True)
                nc.scalar.activation(out=h1_sb[:, m, :], in_=ps[:, 0:512],
                                     func=AF.Gelu, bias=b1_sb[:, m:m + 1],
                                     scale=PSC)

        # FFN matmul 2 (bf16: a second fp8 matmul pushes rel-err past the
        # gate) + bias + residual
        w2r = dram["w2"][:]
        KH2 = DFF // 128            # 32 k-chunks
        w2tags = ["catt8a", "catt8b", "cAB"]
        for m in range(NCH):
            w2t = u.tile([128, KH2, 128], bf, tag=w2tags[m % 3], name=f"w2t_{m}")
            nc.sync.dma_start(out=w2t[:, 0:KH2 // 2, :], in_=w2r[m, 0])
            nc.sync.dma_start(out=w2t[:, KH2 // 2:KH2, :], in_=w2r[m, 1])
            ps = p2(f"ps_h2_{m}")
            for kc in range(KH2):
                nc.tensor.matmul(ps[:, 0:512], lhsT=w2t[:, kc, :],
                                 rhs=h1_sb[:, kc, :],
                                 start=(kc == 0), stop=(kc == KH2 - 1))
            nc.vector.tensor_add(out=os_[m][:], in0=ps[:, 0:512], in1=os_[m][:])
            if not skip_b2:
                nc.vector.tensor_scalar_add(out=os_[m][:], in0=os_[m][:],
                                            scalar1=b2_sb[:, m:m + 1])
            nc.gpsimd.dma_start(
                out=out_d[:].rearrange("(c p) n -> p c n", p=128)[:, m, :],
                in_=os_[m][:])

    _split_sync_waits(nc)
    return nc


# ---------------------------------------------------------------- host side

def _rope_tables(pos, g, b_ln):
    """Feature-major rope coefficient tiles [128, N] (pattern repeats per 64).

    out = (C2*z + S2*rot(z)) * A + Tadd with z the per-head centered vector,
    C2 = C*G[p], S2 = S*G[rp], Tadd = C*B[p] + S*B[rp]. The mean-subtraction
    of the layernorm is folded into the projection weights host-side.
    """
    half = DH // 2
    inv = (1.0 / (10000.0 ** (np.arange(half, dtype=np.float32) / half))).astype(np.float32)
    ang = pos.astype(np.float32)[None, :] * inv[:, None]          # [32, N]
    c = np.cos(ang).astype(np.float32)
    s = np.sin(ang).astype(np.float32)
    C64 = np.concatenate([c, c], axis=0)                          # [64, N]
    S64 = np.concatenate([-s, s], axis=0)
    G = np.ones(DH, np.float32) if g is None else np.asarray(g, np.float32)
    Bv = np.zeros(DH, np.float32) if b_ln is None else np.asarray(b_ln, np.float32)
    rp = np.concatenate([np.arange(32, 64), np.arange(0, 32)])
    C2 = C64 * G[:, None]
    S2 = S64 * G[rp][:, None]
    Tadd = C64 * Bv[:, None] + S64 * Bv[rp][:, None]
    tile = lambda X: np.concatenate([X, X], axis=0)               # [128, N]
    has_t = bool(np.abs(Bv).max() > 0)
    return (tile(C2).astype(BF16), tile(S2).astype(BF16),
            tile(Tadd).astype(BF16) if has_t else None)


def _consts():
    bo16 = np.zeros((128, 8, 16), np.float32)
    for c in range(NCH):
        for pp in range(128):
            bo16[pp, c, 2 * c + (pp >= 64)] = 1.0 / DH
    bo16 = bo16.reshape(128, 8 * 16)
    indall = np.zeros((16, D), np.float32)
    for c in range(NCH):
        for pp in range(128):
            indall[2 * c + (pp >= 64), c * 128 + pp] = 1.0
    perm = np.zeros((128, 128), np.float32)
    for mm in range(128):
        k = (mm // 64) * 64 + ((mm % 64) + 32) % 64
        perm[k, mm] = 1.0
    ind2 = np.zeros((2, 128), np.float32)
    ind2[0, 0:64] = 1.0
    ind2[1, 64:128] = 1.0
    return {
        "bo16": bo16.astype(BF16),
        "indall": indall.astype(BF16),
        "ind2": ind2.astype(BF16),
        "perm": perm.astype(BF16),
        "ones128f": np.full((128, 1), 1.0 / D, np.float32),
        "ones128b": np.full((128, 1), 1.0 / D, BF16),
        "onesr": np.ones((1, 128), BF16),
    }


def _center_heads(w):
    """Project out the per-head mean: w' = w @ blockdiag(I - J/dh). Folding
    this into the QK projections makes the on-chip per-head LN a pure rstd
    scale (the -mu term vanishes)."""
    wr = np.asarray(w, np.float64).reshape(-1, H, DH)
    wr = wr - wr.mean(axis=-1, keepdims=True)
    return wr.reshape(np.asarray(w).shape).astype(np.float32)


def make_in_maps(inputs):
    """Full inputs -> (per-core input dicts, build flags)."""
    inputs = {k: np.asarray(v) for k, v in inputs.items()}
    consts = _consts()
    def tile_w(w, K, Mo):
        # [K*128, Mo*128] -> [Mo, 128(p), K(kc), 128] with w[kc*128+p, m*128+j]
        return np.ascontiguousarray(
            (w * WS).reshape(K, 128, Mo, 128).transpose(2, 1, 0, 3)).astype(E4)

    w2t = inputs["W2"].reshape(2, 16, 128, NCH, 128).transpose(3, 0, 2, 1, 4)
    shared = {
        "wq": tile_w(_center_heads(inputs["Wq"]), NCH, NCH),
        "wk": tile_w(_center_heads(inputs["Wk"]), NCH, NCH),
        "wv": np.ascontiguousarray(
            (inputs["Wv"] * WS).reshape(NCH, 128, D).transpose(1, 0, 2)).astype(E4),
        "w1": np.ascontiguousarray(
            (inputs["W1"] * WS).reshape(NCH, 128, NCH, 4, 128)
            .transpose(2, 1, 0, 3, 4).reshape(NCH, 128, NCH, 512)).astype(E4),
        "w2": np.ascontiguousarray(w2t).astype(BF16),
        "bq": _center_heads(inputs["bq"]), "bk": _center_heads(inputs["bk"]),
        "b1": inputs["b1"].astype(np.float32), "b2": inputs["b2"].astype(np.float32),
        "fng": inputs["fn_g"].astype(np.float32),
        # fnb scaled by XS: h_sb carries the fp8 pre-scale
        "fnb": (inputs["fn_b"] * XS).astype(np.float32),
        **consts,
    }
    in_maps = []
    with_tq = with_tk = False
    for core in range(8):
        b, t0 = core // 4, (core % 4) * T
        xq_slice = np.ascontiguousarray(inputs["query"][b, t0:t0 + T].T).astype(np.float32)
        # the V projection bias is exactly additive after softmax; fold it into
        # the residual here
        xqf = xq_slice + inputs["bv"].astype(np.float32)[:, None]
        cq, sq, tq = _rope_tables(inputs["qpos"][b, t0:t0 + T],
                                  inputs["qn_g"], inputs["qn_b"])
        ck, sk, tk = _rope_tables(inputs["cpos"][b],
                                  inputs["kn_g"], inputs["kn_b"])
        m = dict(shared)
        m.update({
            "xqf": xqf, "xq": (xq_slice * XS).astype(E4),
            "xqfm": xqf.mean(axis=0, keepdims=True).astype(np.float32),
            "xc": np.ascontiguousarray(
                (inputs["context"][b].T * XS)).astype(E4),
            "cq2": cq, "sq2": sq,
            "ck2": ck, "sk2": sk,
        })
        if tq is not None:
            m["tq"] = tq
            with_tq = True
        if tk is not None:
            m["tk"] = tk
            with_tk = True
        in_maps.append(m)
    return in_maps, with_tq, with_tk


def kernel(**inputs):
    _maybe_patch_ldw_opt()
    from concourse.bass_utils import run_bass_kernel_spmd
    in_maps, with_tq, with_tk = make_in_maps(inputs)
    skip_fn = bool(np.all(np.asarray(inputs["fn_g"]) == 1.0)
                   and np.all(np.asarray(inputs["fn_b"]) == 0.0))
    skip_b2 = bool(np.all(np.asarray(inputs["b2"]) == 0.0))
    key = (with_tq, with_tk, skip_fn, skip_b2)
    if key not in _BUILT:
        _BUILT[key] = build(*key)
    nc = _BUILT[key]
    res = run_bass_kernel_spmd(nc, in_maps, core_ids=list(range(8)))
    out = np.zeros((B, N, D), np.float32)
    for core in range(8):
        b, t0 = core // 4, (core % 4) * T
        out[b, t0:t0 + T] = res.results[core]["out"].T
    return out



# revision 54
# speedup vs baseline: 1.2022x; 1.2022x over previous
"""Trainium2 Bass kernel for nn_AttnFuser (fused MHA + FFN transformer block).

Sharding: 8 cores = 2 batches x 4 query-token slices of 512. Each core computes
the full block for its 512 query tokens; K/V projection over the full context
of its batch is replicated within each 4-core batch group (no collectives).

On-chip layout is feature-major ([feature, token]) for Q/K and the FFN, and
token-major for V. The Q/K/V projections and the attention AV matmul run in
fp8e4/fp8e5 with DoubleRow perf mode (2 contraction rows per PE cell -> half
the matmul cycles); operands are pre-scaled (weights x512, activations x16,
V x32) so fp8's narrow mantissa lands on well-conditioned data, and the
scales are folded into the PSUM->SBUF copies / softmax normalization. The
QK matmul and the FFN stay bf16 (fp8 there costs too much accuracy).

The softmax exp over the [2048 x 1024] score block of each head pair is the
single largest elementwise job, so it is split across two engines: ACT
computes true exp into fp8e5, and DVE computes a Schraudolph-style exp for
the other chunks -- y = round(score * 0.5*log2(e) + 62) written as uint8 IS
the fp8e5 bit pattern of ~exp(score/8) (the +62 bias error is a uniform
factor that cancels in softmax; fp32->uint8 saturation maps the negative
tail to +0.0). Softmax denominators come free from a ones-column in V, are
reciprocal'd per head pair, and each pair's normalization + FFN-LN stats
are pipelined into the next pair's attention so nothing serializes at the
attention->FFN boundary.

SBUF is tight, so large tiles share pool tags in strict temporal chains
(e.g. the context tile's slot is later reused by the FFN hidden activations).
"""
import os
import numpy as np
import ml_dtypes

BF16 = ml_dtypes.bfloat16
E4 = ml_dtypes.float8_e4m3
E5 = ml_dtypes.float8_e5m2
XS = 16.0      # fp8 activation pre-scale
WS = 512.0     # fp8 weight pre-scale
VS = 32.0      # V value scale (cancels in softmax normalization)
PSC = 1.0 / (XS * WS)   # PSUM descale for fp8 projections
SCH_A = 0.125 * 4.0 / float(np.log(2.0))   # Schraudolph slope (folds the /8)
SCH_B = 62.0                               # Schraudolph bias (cancels in softmax)


def _maybe_patch_ldw_opt():
    """KERNEL_LDW_OPT=1: flip walrus --enable-ldw-opt to true (dedups/pipelines
    LDWEIGHTS). Verified against the reference on every run."""
    if os.environ.get("KERNEL_LDW_OPT") != "1":
        return
    import concourse.bass_utils as bu
    if getattr(bu, "_ldw_patched", False):
        return
    orig = bu.run_command

    def run_command_ldw(argv, **kw):
        argv = ["--enable-ldw-opt=true" if a == "--enable-ldw-opt=false" else a
                for a in argv]
        return orig(argv, **kw)

    bu.run_command = run_command_ldw
    bu._ldw_patched = True

D, T, M, H, DH, DFF = 1024, 512, 2048, 16, 64, 4096
NCH = D // 128      # 8 feature chunks
TTK = M // 512      # 4 context token tiles
MC = M // 128       # 16 context chunks
B, N = 2, 2048      # full problem dims

_BUILT = {}


def _patch_tile_drain():
    """This walrus build rejects >1 sem wait on an InstDrain (TPB_CTRL
    setupSyncWait). Split the TileContext tail-drain waits onto nop insts."""
    import concourse.tile as tile_mod
    from concourse import mybir
    from concourse.vector_clock import ScopedClock
    if getattr(tile_mod.TileContext, "_drain_patched", False):
        return

    def _drain_and_barrier(self, tick_clock, wait_clock):
        nc = self.nc
        drain_inst = nc.sync.drain()
        wait_clock.add_sem_waits(
            drain_inst.ins, ScopedClock({None: tick_clock.global_clock}))
        si = drain_inst.ins.sync_info
        waits = list(si.on_wait or []) if si else []
        if len(waits) > 1:
            drain_inst.ins.sync_info = mybir.SyncInfo(
                on_wait=waits[:1], on_update=list(si.on_update or []))
            for w in waits[1:]:
                nop = nc.sync.nop(nofuse=True, hint="split_drain_wait")
                nop.ins.sync_info = mybir.SyncInfo(on_wait=[w], on_update=[])
        nc.all_engine_barrier()
        popped = nc._tile_sem_poison_stack.pop()
        assert popped is self._sem_poison
        nc.clear_and_free_semaphores(list(self.sems.allocated().values()))
        nc.all_engine_barrier()

    tile_mod.TileContext._drain_and_barrier = _drain_and_barrier
    tile_mod.TileContext._drain_patched = True


def _split_sync_waits(nc, max_waits=1):
    """This walrus build rejects instructions carrying more than ~1 sem wait
    (setupSyncWait: 'Too many sync wait commands'). Hoist extra waits onto
    same-engine NOPs inserted immediately before the instruction — the engine
    executes them in order, so all waits are still satisfied before the op."""
    from concourse import mybir
    n = 0
    for f in nc.m.functions:
        for bb in f.blocks:
            insts = bb.instructions
            new = []
            for inst in insts:
                si = getattr(inst, "sync_info", None)
                waits = list(si.on_wait) if si and si.on_wait else []
                if len(waits) > max_waits:
                    for w in waits[max_waits:]:
                        nop = mybir.InstNoOp(
                            name=f"wsplit_{n}",
                            sync_info=mybir.SyncInfo(on_wait=[w], on_update=[]),
                            bass_nofuse=True,
                            engine=inst.engine,
                        )
                        nc.register_instruction(nop)
                        n += 1
                        new.append(nop)
                    inst.sync_info = mybir.SyncInfo(
                        on_wait=waits[:max_waits],
                        on_update=list(si.on_update or []))
                new.append(inst)
            insts[:] = new
    return n


def input_specs(with_tq, with_tk, skip_fn=False, skip_b2=False):
    """(name, shape, np_dtype) for every DRAM input."""
    sp = [
        ("xq", (D, T), E4), ("xqf", (D, T), np.float32),
        ("xc", (D, M), E4),
        ("wq", (NCH, 128, NCH, 128), E4), ("wk", (NCH, 128, NCH, 128), E4),
        ("wv", (128, NCH, D), E4),
        ("w1", (NCH, 128, NCH, 512), E4),
        ("w2", (NCH, 2, 128, DFF // 256, 128), BF16),
        ("bq", (D,), np.float32), ("bk", (D,), np.float32),
        ("b1", (DFF,), np.float32), ("b2", (D,), np.float32),
        ("fng", (D,), np.float32), ("fnb", (D,), np.float32),
        ("cq2", (128, T), BF16), ("sq2", (128, T), BF16),
        ("ck2", (128, M), BF16), ("sk2", (128, M), BF16),
        ("bo16", (128, 8 * 16), BF16),
        ("indall", (16, D), BF16),
        ("ind2", (2, 128), BF16),
        ("perm", (128, 128), BF16),
        ("id128", (128, 128), BF16),
        ("ones128f", (128, 1), np.float32),  # value 1/1024 (FFN stats lhsT)
        ("ones128b", (128, 1), BF16),        # bf16 twin for bf16 stats matmuls
        ("xqfm", (1, T), np.float32),        # feature-mean of xqf per token
        ("onesr", (1, 128), BF16),           # bf16 ones row (FFN bc lhsT)
    ]
    if with_tq:
        sp.append(("tq", (128, T), BF16))
    if with_tk:
        sp.append(("tk", (128, M), BF16))
    return sp


def build(with_tq=False, with_tk=False, skip_fn=False, skip_b2=False):
    import concourse.bass as bass
    import concourse.mybir as mybir
    import concourse.tile as tile
    from contextlib import ExitStack

    _patch_tile_drain()
    AF = mybir.ActivationFunctionType
    OP = mybir.AluOpType
    DR = mybir.MatmulPerfMode.DoubleRow
    bf = mybir.dt.bfloat16
    f32 = mybir.dt.float32
    f8 = mybir.dt.float8e4
    f8e5 = mybir.dt.float8e5
    u8 = mybir.dt.uint8

    nc = bass.Bass()
    dram = {}
    for name, shape, npdt in input_specs(with_tq, with_tk, skip_fn, skip_b2):
        dt = {BF16: bf, E4: f8, E5: f8e5}.get(npdt, f32)
        dram[name] = nc.declare_dram_parameter(name, list(shape), dt, isOutput=False)
    out_d = nc.declare_dram_parameter("out", [D, T], f32, isOutput=True)

    with tile.TileContext(nc) as tc, ExitStack() as ctx:
        const = ctx.enter_context(tc.tile_pool(name="const", bufs=1))
        u = ctx.enter_context(tc.tile_pool(name="u", bufs=1))
        wt = ctx.enter_context(tc.tile_pool(name="wt", bufs=2))
        # PSUM: 2 x 2-bank tiles (scores/proj/FFN) + 2 x 1-bank (rope) +
        # 2 x 1-bank (attention O accumulators / LN stats) = 8 banks total
        p2p = ctx.enter_context(tc.tile_pool(name="p2p", bufs=2, space="PSUM"))
        prp = ctx.enter_context(tc.tile_pool(name="prp", bufs=2, space="PSUM"))
        pot = ctx.enter_context(tc.tile_pool(name="pot", bufs=2, space="PSUM"))

        def p2(name):
            return p2p.tile([128, 1024], f32, tag="p2", name=name)

        def pr(name):
            return prp.tile([128, 512], f32, tag="pr", name=name)

        def load(pool, name, rearr=None, tag=None, eng=None, **kw):
            src = dram[name][:]
            if rearr is not None:
                src = src.rearrange(rearr, **kw)
            t = pool.tile(list(src.shape), src.dtype, tag=tag or name, name=name)
            (eng or nc.gpsimd).dma_start(out=t[:], in_=src)
            return t

        # -------- DMA front-load: what the first ~40us needs, in need order.
        # sync queue: xq then the wq stream (issued inside proj); gpsimd+scalar
        # split the context, then wv, then the small constants.
        xc_sb = u.tile([128, NCH, M], f8, tag="cA", name="xc")       # cA: xc->h1
        xq_sb = u.tile([128, NCH, T], f8, tag="cE", name="xqbf")     # cE: xq->h
        nc.sync.dma_start(out=xq_sb[:], in_=dram["xq"][:].rearrange("(c p) n -> p c n", p=128))
        bq_sb = load(const, "bq", "(c p) -> p c", p=128)
        bk_sb = load(const, "bk", "(c p) -> p c", p=128, eng=nc.scalar)
        xcr = dram["xc"][:].rearrange("(c p) m -> p c m", p=128)
        engs = [nc.gpsimd, nc.scalar]
        for c in range(NCH):
            engs[c % 2].dma_start(out=xc_sb[:, c, :], in_=xcr[:, c, :])
        wv_sb = u.tile([128, NCH, D], f8, tag="cD", name="wvsb")     # cD: wv->OT
        nc.gpsimd.dma_start(out=wv_sb[:, 0:4, :], in_=dram["wv"][:][:, 0:4, :])
        nc.scalar.dma_start(out=wv_sb[:, 4:8, :], in_=dram["wv"][:][:, 4:8, :])

        bo16 = load(const, "bo16")
        indall = load(const, "indall", eng=nc.scalar)
        ind2 = load(const, "ind2")
        perm = load(const, "perm", eng=nc.scalar)
        id128 = load(const, "id128", eng=nc.scalar)
        ones128f = load(const, "ones128f")
        ones128b = load(const, "ones128b")
        xqfm_sb = load(const, "xqfm")
        onesr = load(const, "onesr", eng=nc.scalar)
        eps = const.tile([128, 1], f32, tag="eps", name="eps")
        nc.vector.memset(eps[:], 1e-5)
        # af bias = ln(XS): folds the fp8 activation pre-scale for the FFN
        # input into the LN rstd (h_sb is stored fp8e4 scaled by XS)
        lnxs = const.tile([1, 1], f32, tag="lnxs", name="lnxs")
        nc.vector.memset(lnxs[:], float(np.log(XS)))
        salt = int(os.environ.get("KERNEL_SALT", "0"))
        if salt:
            # cache-busting dummy (changes BIR bytes so the NEFF cache misses)
            dummy = const.tile([1, 1], f32, tag="dummy", name="dummy")
            nc.vector.memset(dummy[:], float(salt))

        # big activations (tags are temporal chains -- comments show the chain)
        # K is per-chunk so each o_c can take over chunk c's slot right after
        # pair c's QK matmuls (a single KT tile would hold its slot until
        # pair 7 and deadlock the per-pair normalization pipeline)
        KTs = [u.tile([128, M], bf, tag=f"cKT{c}", name=f"KT{c}")
               for c in range(NCH)]
        QT = u.tile([128, NCH, T], bf, tag="cQT", name="QT")
        V = u.tile([128, MC, H, DH + 1], f8, tag="cC", name="V")     # cC: V->x2f
        aK = u.tile([16, M], bf, tag="caK", name="aK")               # caK: aK->A2
        aQ = u.tile([16, T], bf, tag="caQ", name="aQ")               # caQ: aQ->af
        xqf_sb = u.tile([128, NCH, T], f32, tag="cF", name="xqf")

        # V ones-columns (col DH for every head); value VS cancels against the
        # VS scale of O in the softmax normalization
        nc.vector.memset(V[:, :, :, DH:DH + 1], VS)

        # ---------------- projections (fp8 DoubleRow) ----------------
        def proj_featmajor(wname, x_sb, ntt, outs, bias):
            wr = dram[wname][:]
            npairs = (ntt + 1) // 2
            for m in range(NCH):
                wtile = wt.tile([128, NCH, 128], f8, tag="wqk", name=f"w_{wname}_{m}")
                nc.sync.dma_start(out=wtile[:], in_=wr[m])
                pts = [p2(f"ps_{wname}_{m}_{j}") for j in range(npairs)]
                halves = [pts[n // 2][:, (n % 2) * 512:(n % 2) * 512 + 512]
                          for n in range(ntt)]
                for kc in range(NCH // 2):
                    for n in range(ntt):
                        nc.tensor.matmul(halves[n], lhsT=wtile[:, 2 * kc:2 * kc + 2, :],
                                         rhs=x_sb[:, 2 * kc:2 * kc + 2, n * 512:(n + 1) * 512],
                                         start=(kc == 0), stop=(kc == NCH // 2 - 1),
                                         perf_mode=DR, skip_group_check=True)
                for j in range(npairs):
                    wid = min(1024, (ntt - 2 * j) * 512)
                    nc.scalar.activation(out=outs[m][:, 1024 * j:1024 * j + wid],
                                         in_=pts[j][:, 0:wid], func=AF.Identity,
                                         bias=bias[:, m:m + 1], scale=PSC)

        # ---------------- per-head QK layernorm + rope ----------------
        # The per-head mean is projected out of Wq/Wk host-side, so LN reduces
        # to a pure rstd scale: X = (C2*x + S2*rot(x)) * A [+ Tadd].
        def ln_sq_stats(Xs, ntt, a_sb, pref):
            # sq PSUM rides the pot ring (idle until attention), so the stats
            # never contend with projection/score tiles in the p2 ring
            for tt in range(ntt):
                ts_ = slice(tt * 512, (tt + 1) * 512)
                sqp = pot.tile([16, 512], f32, tag="pot", name=f"sq_{pref}{tt}")
                sq = sqp[0:16, 0:512]
                for c in range(NCH):
                    xs = Xs[c][:, ts_]
                    x2 = u.tile([128, 512], bf, tag="x2", bufs=2, name=f"x2_{pref}{tt}_{c}")
                    # squares on GpSimd: DVE activity halves the PE clock
                    # (HAM activity throttle), and GpSimd is otherwise idle
                    nc.gpsimd.tensor_mul(out=x2[:], in0=xs, in1=xs)
                    # one-hot block lhsT accumulates chunk c's two head rows
                    nc.tensor.matmul(sq, lhsT=bo16[:, c * 16:(c + 1) * 16], rhs=x2[:],
                                     start=(c == 0), stop=(c == NCH - 1),
                                     skip_group_check=True)
                # rstd = exp(-0.5*ln(E[x^2]+eps)): two ACT table ops straight
                # from PSUM (inputs are centered, so E[x^2] is the variance)
                lnv = u.tile([16, 512], f32, tag="csd", name=f"lnv_{pref}{tt}")
                nc.scalar.activation(out=lnv[:], in_=sq, func=AF.Ln,
                                     bias=eps[0:16, :], scale=1.0)
                with nc.allow_low_precision("bf16 rstd for broadcast matmul"):
                    nc.scalar.activation(out=a_sb[:, ts_], in_=lnv[:], func=AF.Exp,
                                         scale=-0.5)

        def ln_rope(Xs, ntt, a_sb, ctab, stab, ttab, pref, only_c=None):
            # per-512-token slices through the 1-bank prp ring; the aps
            # broadcast is emitted right before its single consumer (a DVE op
            # can read at most one PSUM operand, so the add stays on DVE)
            for c in (range(NCH) if only_c is None else [only_c]):
                for tt in range(ntt):
                    ts_ = slice(tt * 512, (tt + 1) * 512)
                    rot = pr(f"rot_{pref}{c}_{tt}")
                    nc.tensor.matmul(rot[:, 0:512], lhsT=perm[:], rhs=Xs[c][:, ts_],
                                     start=True, stop=True, skip_group_check=True)
                    zz1 = u.tile([128, 512], bf, tag="czz1", bufs=2,
                                 name=f"zz1_{pref}{c}_{tt}")
                    zz2 = u.tile([128, 512], bf, tag="czz2", bufs=2,
                                 name=f"zz2_{pref}{c}_{tt}")
                    nc.vector.tensor_mul(out=zz2[:], in0=rot[:, 0:512],
                                         in1=stab[:, ts_])
                    nc.vector.tensor_mul(out=zz1[:], in0=Xs[c][:, ts_],
                                         in1=ctab[:, ts_])
                    nc.vector.tensor_add(out=zz1[:], in0=zz1[:], in1=zz2[:])
                    aps = pr(f"aps_{pref}{c}_{tt}")
                    nc.tensor.matmul(aps[:, 0:512], lhsT=indall[:, c * 128:(c + 1) * 128],
                                     rhs=a_sb[:, ts_], start=True, stop=True,
                                     skip_group_check=True)
                    if ttab is None:
                        nc.vector.tensor_mul(out=Xs[c][:, ts_], in0=zz1[:],
                                             in1=aps[:, 0:512])
                    else:
                        nc.vector.tensor_mul(out=zz1[:], in0=zz1[:], in1=aps[:, 0:512])
                        nc.vector.tensor_add(out=Xs[c][:, ts_], in0=zz1[:],
                                             in1=ttab[:, ts_])

        QTs = [QT[:, c, :] for c in range(NCH)]
        proj_featmajor("wq", xq_sb, 1, QTs, bq_sb)
        cq2 = load(u, "cq2", eng=nc.sync); sq2 = load(u, "sq2", eng=nc.sync)
        tqt = load(const, "tq") if with_tq else None
        tkt = load(const, "tk") if with_tk else None
        # K projection next: a dense PE burst that keeps the ramped clock
        # while DVE chews the Q stats + rope emitted right after
        proj_featmajor("wk", xc_sb, TTK, [t[:] for t in KTs], bk_sb)
        ck2 = load(u, "ck2", eng=nc.sync); sk2 = load(u, "sk2", eng=nc.sync)
        ln_sq_stats(QTs, 1, aQ, "q")
        ln_rope(QTs, 1, aQ, cq2, sq2, tqt, "q")

        def v_proj_pair(g):
            # two context chunks of the V projection (PE work that overlaps
            # the DVE-bound K layernorm+rope); fp8 DoubleRow over kc pairs
            for mc in (2 * g, 2 * g + 1):
                ps = p2(f"ps_v_{mc}")
                for kc in range(NCH // 2):
                    lh = xc_sb[:, 2 * kc:2 * kc + 2, mc * 128:(mc + 1) * 128]
                    nc.tensor.matmul(ps[:, 0:512], lhsT=lh,
                                     rhs=wv_sb[:, 2 * kc:2 * kc + 2, 0:512],
                                     start=(kc == 0), stop=(kc == NCH // 2 - 1),
                                     perf_mode=DR, skip_group_check=True)
                    nc.tensor.matmul(ps[:, 512:1024], lhsT=lh,
                                     rhs=wv_sb[:, 2 * kc:2 * kc + 2, 512:1024],
                                     start=(kc == 0), stop=(kc == NCH // 2 - 1),
                                     perf_mode=DR, skip_group_check=True)
                pv = ps[:].rearrange("p (hh d) -> p hh d", d=DH)
                nc.scalar.activation(out=V[:, mc, 0:H, 0:DH], in_=pv[:],
                                     func=AF.Copy, scale=PSC * VS)

        # xqf (residual) loads late (first use is pair-0 normalization, inside
        # pair 2 of the attention); OT takes over wv's slot
        nc.sync.dma_start(out=xqf_sb[:], in_=dram["xqf"][:].rearrange("(c p) n -> p c n", p=128))
        OT = u.tile([128, NCH, T], bf, tag="cD", name="OT")

        # V projection runs upfront (a dense PE burst that overlaps the
        # DVE-bound Q-rope tail); attention pair 0's AV needs all of V
        for g in range(NCH):
            v_proj_pair(g)
        ln_sq_stats(KTs, TTK, aK, "k")
        # w1 group 0/1 prefetch into dedicated slots: the DMA queue is idle
        # here and the shared att-slot copies couldn't start until the last
        # attention wave drained
        w1r = dram["w1"][:]
        w1g01 = u.tile([128, 2, NCH, 512], f8, tag="cw1p", name="w1g01")
        nc.sync.dma_start(out=w1g01[:, 0], in_=w1r[0])
        nc.sync.dma_start(out=w1g01[:, 1], in_=w1r[1])

        # ---------------- fused K-rope + attention ----------------
        # Attention pair p only needs rope'd K chunk p, so pair p-1 rides
        # right behind rope chunk p: the PE stream alternates [rope-PE(c),
        # QK/AV(c-1)] in long runs (keeps the HAM clock-gate at 8/8), while
        # DVE/ACT run rope elementwise + the exp split concurrently.
        # Head pairs: the two K=64 QK matmuls run as concurrent PE row-tiles
        # (rows 0-63 / 64-127) into the two banks of one PSUM tile. The exp of
        # each [128,1024] score chunk splits ACT (true exp) / DVE (Schraudolph
        # uint8 -> fp8e5 bits). Pair c's normalization, residual add and
        # FFN-LN stat accumulation run inside pair c+1's instruction stream so
        # the reciprocal latency hides under attention matmuls.
        WV = 4                      # ctx chunks per wave
        os_ = [None] * NCH
        # x2f rides in the context slot (cA, dead after the V projection) --
        # NOT in V's slot: V is read by every pair's AV matmul, and an aliased
        # write here would stall the in-order ACT queue into a cycle
        x2f = u.tile([128, NCH, T], bf, tag="cA", name="x2f")
        acc = u.tile([1, 1024], f32, tag="cacc", name="acc")   # [mean | meansq]
        dall2s = [None] * NCH

        def norm_pair(c):
            # softmax-normalize chunk c (heads 2c/2c+1), residual, FFN-LN stats
            os_[c] = u.tile([128, T], f32, tag=f"cKT{c}", name=f"o_{c}")
            xs2 = x2f[:, c, :]
            rt = u.tile([2, 512], f32, tag="crt", bufs=1, name=f"rt_{c}")
            nc.scalar.activation(out=rt[:], in_=dall2s[c][:], func=AF.Ln)
            rl = u.tile([2, 512], bf, tag="crl", bufs=1, name=f"rl_{c}")
            with nc.allow_low_precision("bf16 softmax reciprocal broadcast"):
                nc.scalar.activation(out=rl[:], in_=rt[:], func=AF.Exp, scale=-1.0)
            rps = pr(f"rps_{c}")
            nc.tensor.matmul(rps[:, 0:512], lhsT=ind2[:], rhs=rl[:],
                             start=True, stop=True)
            rsb = u.tile([128, 512], bf, tag="crsb", bufs=2, name=f"rsb_{c}")
            nc.scalar.activation(out=rsb[:], in_=rps[:, 0:512], func=AF.Copy)
            nc.vector.tensor_mul(out=OT[:, c, :], in0=OT[:, c, :], in1=rsb[:])
            nc.vector.tensor_add(out=os_[c][:], in0=xqf_sb[:, c, :],
                                 in1=OT[:, c, :])
            with nc.allow_low_precision("bf16 squares for FFN-LN variance"):
                nc.vector.tensor_mul(out=xs2, in0=os_[c][:], in1=os_[c][:])
            # stats in bf16 (fp32 matmuls run LOW_HIGH at 4x the cycles); the
            # residual's feature-mean is added from a host-computed row later
            smm = p2(f"smm_{c}")
            nc.tensor.matmul(smm[0:1, 0:512], lhsT=ones128b[:], rhs=OT[:, c, :],
                             start=True, stop=True, skip_group_check=True)
            nc.tensor.matmul(smm[0:1, 512:1024], lhsT=ones128b[:], rhs=xs2,
                             start=True, stop=True, skip_group_check=True)
            if c == 0:
                nc.vector.tensor_copy(out=acc[:], in_=smm[0:1, :])
            else:
                nc.vector.tensor_add(out=acc[:], in0=acc[:], in1=smm[0:1, :])

        def attn_pair(pair):
            he, ho = 2 * pair, 2 * pair + 1
            c = pair
            ote = pot.tile([128, 512], f32, tag="pot", name=f"ot_{he}")
            oto = pot.tile([128, 512], f32, tag="pot", name=f"ot_{ho}")
            for w in range(MC // WV):
                # waves 0-2: ACT true exp into fp8e4 (scores exp(s/8) stay
                # well under e4m3's 448 max, and e4m3 quantizes 4x finer than
                # e5m2); wave 3 carries the DVE Schraudolph chunks, whose
                # uint8 bit-trick is e5m2-only
                att = u.tile([128, WV, 1024], f8 if w < 3 else f8e5,
                             tag=("catt8a" if w % 2 == 0 else "catt8b"),
                             name=f"att_{pair}_{w}")
                for i in range(WV):
                    mc = w * WV + i
                    sp = p2(f"sp_{pair}_{mc}")
                    nc.tensor.matmul(sp[:, 0:512],
                                     lhsT=KTs[c][0:64, mc * 128:(mc + 1) * 128],
                                     rhs=QT[0:64, c, :], start=True, stop=True,
                                     tile_position=(0, 0))
                    nc.tensor.matmul(sp[:, 512:1024],
                                     lhsT=KTs[c][64:128, mc * 128:(mc + 1) * 128],
                                     rhs=QT[64:128, c, :], start=True, stop=True,
                                     tile_position=(64, 0))
                    if mc < 14:
                        with nc.allow_low_precision("fp8 softmax weights"):
                            nc.scalar.activation(out=att[:, i, :], in_=sp[:],
                                                 func=AF.Exp, scale=0.125)
                    else:
                        attu = att[:, i, :].bitcast(u8)
                        with nc.allow_low_precision("schraudolph exp to fp8e5 bits"):
                            nc.vector.tensor_scalar(out=attu, in0=sp[:],
                                                    scalar1=SCH_A, scalar2=SCH_B,
                                                    op0=OP.mult, op1=OP.add)
                for i2 in range(WV // 2):
                    mcp = w * WV + 2 * i2
                    nc.tensor.matmul(ote[0:DH + 1, :],
                                     lhsT=V[:, mcp:mcp + 2, he, :],
                                     rhs=att[:, 2 * i2:2 * i2 + 2, 0:512],
                                     start=(mcp == 0), stop=(mcp == MC - 2),
                                     perf_mode=DR, skip_group_check=True)
                    nc.tensor.matmul(oto[0:DH + 1, :],
                                     lhsT=V[:, mcp:mcp + 2, ho, :],
                                     rhs=att[:, 2 * i2:2 * i2 + 2, 512:1024],
                                     start=(mcp == 0), stop=(mcp == MC - 2),
                                     perf_mode=DR, skip_group_check=True)
                if w == 2 and pair >= 1:
                    # previous pair's softmax normalization rides here: one
                    # extra wave of slack keeps the reciprocal's PE broadcast
                    # matmul from stalling the in-order PE stream
                    norm_pair(pair - 1)
            # stash unnormalized O and the denominators (row 64 = ones-column)
            dall2 = u.tile([2, 512], bf, tag="cdall", bufs=1, name=f"dall2_{pair}")
            dall2s[pair] = dall2
            # rows 0..63 = O, row 64 = denominator (ones-column): one copy
            # covers both; the denominator row DMAs out before the odd head's
            # stash overwrites partitions 64+. Copies ride ACT (off DVE).
            with nc.allow_low_precision("bf16 O stash"):
                nc.scalar.activation(out=OT[0:65, c, :], in_=ote[0:65, :],
                                     func=AF.Copy)
            nc.sync.dma_start(out=dall2[0:1, :], in_=OT[64:65, c, :])
            # odd head: O sits at PSUM rows 0..63 but belongs at partitions
            # 64..127 of OT; shift with a bounce through SBUF + gpsimd DMA
            tmp = u.tile([128, 512], bf, tag="cotmp", bufs=1, name=f"otmp_{ho}")
            with nc.allow_low_precision("bf16 O stash"):
                nc.scalar.activation(out=tmp[0:65, :], in_=oto[0:65, :],
                                     func=AF.Copy)
            nc.gpsimd.dma_start(out=OT[64:128, c, :], in_=tmp[0:64, :])
            nc.gpsimd.dma_start(out=dall2[1:2, :], in_=tmp[64:65, :])

        # fused K-rope + attention: rope chunk c's PE/DVE work interleaves
        # with attention pair c-1, keeping the PE stream dense (p-state) while
        # DVE alternates rope and Schraudolph/normalization work
        ln_rope(KTs, TTK, aK, ck2, sk2, tkt, "k", only_c=0)
        for c in range(1, NCH):
            ln_rope(KTs, TTK, aK, ck2, sk2, tkt, "k", only_c=c)
            attn_pair(c - 1)
        attn_pair(NCH - 1)
        norm_pair(NCH - 1)

        # ---------------- FFN ----------------
        b2_sb = load(const, "b2", "(c p) -> p c", p=128)
        b1_sb = load(const, "b1", "(c p) -> p c", p=128)
        fng_sb = load(const, "fng", "(c p) -> p c", p=128)
        fnb_sb = load(const, "fnb", "(c p) -> p c", p=128)
        mu0 = u.tile([1, 512], f32, tag="cmu", name="mu0")
        nc.vector.tensor_add(out=mu0[:], in0=acc[0:1, 0:512], in1=xqfm_sb[:])
        muf = mu0[:]
        t1f = u.tile([1, 512], f32, tag="ct1", name="t1f")
        nc.vector.tensor_mul(out=t1f[:], in0=muf, in1=muf)
        varf = u.tile([1, 512], f32, tag="cvar", name="varf")
        nc.vector.tensor_tensor(out=varf[:], in0=acc[0:1, 512:1024], in1=t1f[:],
                                op=OP.subtract)
        lnf = u.tile([1, 512], f32, tag="csd", name="lnf")
        nc.scalar.activation(out=lnf[:], in_=varf[:], func=AF.Ln, bias=eps[0:1, :],
                             scale=1.0)
        af = u.tile([1, 512], bf, tag="caQ", name="af")
        with nc.allow_low_precision("bf16 rstd for broadcast matmul"):
            # bias=ln(XS): af = XS * rstd, so h_sb comes out pre-scaled for fp8
            nc.scalar.activation(out=af[:], in_=lnf[:], func=AF.Exp, scale=-0.5,
                                 bias=lnxs[:])
        bff = u.tile([1, 512], bf, tag="cbQ", name="bff")
        nc.vector.tensor_mul(out=bff[:], in0=muf, in1=af[:])
        abf = p2("abf")
        nc.tensor.matmul(abf[:, 0:512], lhsT=onesr[:], rhs=af[:], start=True, stop=True)
        nc.tensor.matmul(abf[:, 512:1024], lhsT=onesr[:], rhs=bff[:], start=True, stop=True)
        A2 = u.tile([128, 512], bf, tag="caK", name="A2")
        nc.scalar.activation(out=A2[:], in_=abf[:, 0:512], func=AF.Copy)
        B2 = u.tile([128, 512], bf, tag="cbK", name="B2")
        nc.scalar.activation(out=B2[:], in_=abf[:, 512:1024], func=AF.Copy)

        h_sb = u.tile([128, NCH, T], f8, tag="cE", name="hsb")
        for c in range(NCH):
            with nc.allow_low_precision("fp8 FFN activations"):
                if skip_fn:
                    tn = u.tile([128, 512], f32, tag="ck2", name=f"tn_{c}")
                    nc.vector.tensor_mul(out=tn[:], in0=os_[c][:], in1=A2[:])
                    nc.vector.tensor_tensor(out=h_sb[:, c, :], in0=tn[:], in1=B2[:],
                                            op=OP.subtract)
                else:
                    tn = u.tile([128, 512], f32, tag="ck2", name=f"tn_{c}")
                    nc.vector.tensor_mul(out=tn[:], in0=os_[c][:], in1=A2[:])
                    nc.vector.tensor_tensor(out=tn[:], in0=tn[:], in1=B2[:], op=OP.subtract)
                    nc.vector.tensor_scalar(out=h_sb[:, c, :], in0=tn[:],
                                            scalar1=fng_sb[:, c:c + 1],
                                            scalar2=fnb_sb[:, c:c + 1],
                                            op0=OP.mult, op1=OP.add)

        # FFN matmul 1 (fp8 DoubleRow) + exact GELU (weights streamed as
        # 0.5MB group tiles through the attention att-tile slots); the
        # 1/(XS*WS) descale folds into gelu's input scale, so h1 is the true
        # gelu output, stored fp8e4 (range ~[-0.17, 6] fits e4m3 natively)
        h1_sb = u.tile([128, DFF // 128, T], bf, tag="cA", name="h1")
        for g in range(NCH):
            if g < 2:
                w1g = w1g01[:, g]
            else:
                w1g = u.tile([128, NCH, 512], f8,
                             tag=("catt8a" if g % 2 == 0 else "catt8b"),
                             name=f"w1g_{g}")
                nc.sync.dma_start(out=w1g[:], in_=w1r[g])
            for mm in range(4):
                m = 4 * g + mm
                ps = p2(f"ps_h1_{m}")
                for kc in range(NCH // 2):
                    nc.tensor.matmul(ps[:, 0:512],
                                     lhsT=w1g[:, 2 * kc:2 * kc + 2,
                                              mm * 128:(mm + 1) * 128],
                                     rhs=h_sb[:, 2 * kc:2 * kc + 2, :],
                                     start=(kc == 0), stop=(kc == NCH // 2 - 1),
                                     perf_mode=DR, skip_group_check=True)
                nc.scalar.activation(out=h1_sb[:, m, :], in_=ps[:, 0:512],
                                     func=AF.Gelu, bias=b1_sb[:, m:m + 1],
                                     scale=PSC)

        # FFN matmul 2 (bf16: a second fp8 matmul pushes rel-err past the
        # gate) + bias + residual
        w2r = dram["w2"][:]
        KH2 = DFF // 128            # 32 k-chunks
        w2tags = ["catt8a", "catt8b", "cAB"]
        for m in range(NCH):
            w2t = u.tile([128, KH2, 128], bf, tag=w2tags[m % 3], name=f"w2t_{m}")
            nc.sync.dma_start(out=w2t[:, 0:KH2 // 2, :], in_=w2r[m, 0])
            nc.sync.dma_start(out=w2t[:, KH2 // 2:KH2, :], in_=w2r[m, 1])
            ps = p2(f"ps_h2_{m}")
            for kc in range(KH2):
                nc.tensor.matmul(ps[:, 0:512], lhsT=w2t[:, kc, :],
                                 rhs=h1_sb[:, kc, :],
                                 start=(kc == 0), stop=(kc == KH2 - 1))
            nc.vector.tensor_add(out=os_[m][:], in0=ps[:, 0:512], in1=os_[m][:])
            if not skip_b2:
                nc.vector.tensor_scalar_add(out=os_[m][:], in0=os_[m][:],
                                            scalar1=b2_sb[:, m:m + 1])
            nc.gpsimd.dma_start(
                out=out_d[:].rearrange("(c p) n -> p c n", p=128)[:, m, :],
                in_=os_[m][:])

    _split_sync_waits(nc)
    return nc


# ---------------------------------------------------------------- host side

def _rope_tables(pos, g, b_ln):
    """Feature-major rope coefficient tiles [128, N] (pattern repeats per 64).

    out = (C2*z + S2*rot(z)) * A + Tadd with z the per-head centered vector,
    C2 = C*G[p], S2 = S*G[rp], Tadd = C*B[p] + S*B[rp]. The mean-subtraction
    of the layernorm is folded into the projection weights host-side.
    """
    half = DH // 2
    inv = (1.0 / (10000.0 ** (np.arange(half, dtype=np.float32) / half))).astype(np.float32)
    ang = pos.astype(np.float32)[None, :] * inv[:, None]          # [32, N]
    c = np.cos(ang).astype(np.float32)
    s = np.sin(ang).astype(np.float32)
    C64 = np.concatenate([c, c], axis=0)                          # [64, N]
    S64 = np.concatenate([-s, s], axis=0)
    G = np.ones(DH, np.float32) if g is None else np.asarray(g, np.float32)
    Bv = np.zeros(DH, np.float32) if b_ln is None else np.asarray(b_ln, np.float32)
    rp = np.concatenate([np.arange(32, 64), np.arange(0, 32)])
    C2 = C64 * G[:, None]
    S2 = S64 * G[rp][:, None]
    Tadd = C64 * Bv[:, None] + S64 * Bv[rp][:, None]
    tile = lambda X: np.concatenate([X, X], axis=0)               # [128, N]
    has_t = bool(np.abs(Bv).max() > 0)
    return (tile(C2).astype(BF16), tile(S2).astype(BF16),
            tile(Tadd).astype(BF16) if has_t else None)


def _consts():
    bo16 = np.zeros((128, 8, 16), np.float32)
    for c in range(NCH):
        for pp in range(128):
            bo16[pp, c, 2 * c + (pp >= 64)] = 1.0 / DH
    bo16 = bo16.reshape(128, 8 * 16)
    indall = np.zeros((16, D), np.float32)
    for c in range(NCH):
        for pp in range(128):
            indall[2 * c + (pp >= 64), c * 128 + pp] = 1.0
    perm = np.zeros((128, 128), np.float32)
    for mm in range(128):
        k = (mm // 64) * 64 + ((mm % 64) + 32) % 64
        perm[k, mm] = 1.0
    ind2 = np.zeros((2, 128), np.float32)
    ind2[0, 0:64] = 1.0
    ind2[1, 64:128] = 1.0
    return {
        "bo16": bo16.astype(BF16),
        "indall": indall.astype(BF16),
        "ind2": ind2.astype(BF16),
        "perm": perm.astype(BF16),
        "id128": np.eye(128, dtype=np.float32).astype(BF16),
        "ones128f": np.full((128, 1), 1.0 / D, np.float32),
        "ones128b": np.full((128, 1), 1.0 / D, BF16),
        "onesr": np.ones((1, 128), BF16),
    }


def _center_heads(w):
    """Project out the per-head mean: w' = w @ blockdiag(I - J/dh). Folding
    this into the QK projections makes the on-chip per-head LN a pure rstd
    scale (the -mu term vanishes)."""
    wr = np.asarray(w, np.float64).reshape(-1, H, DH)
    wr = wr - wr.mean(axis=-1, keepdims=True)
    return wr.reshape(np.asarray(w).shape).astype(np.float32)


def make_in_maps(inputs):
    """Full inputs -> (per-core input dicts, build flags)."""
    inputs = {k: np.asarray(v) for k, v in inputs.items()}
    consts = _consts()
    def tile_w(w, K, Mo):
        # [K*128, Mo*128] -> [Mo, 128(p), K(kc), 128] with w[kc*128+p, m*128+j]
        return np.ascontiguousarray(
            (w * WS).reshape(K, 128, Mo, 128).transpose(2, 1, 0, 3)).astype(E4)

    w2t = inputs["W2"].reshape(2, 16, 128, NCH, 128).transpose(3, 0, 2, 1, 4)
    shared = {
        "wq": tile_w(_center_heads(inputs["Wq"]), NCH, NCH),
        "wk": tile_w(_center_heads(inputs["Wk"]), NCH, NCH),
        "wv": np.ascontiguousarray(
            (inputs["Wv"] * WS).reshape(NCH, 128, D).transpose(1, 0, 2)).astype(E4),
        "w1": np.ascontiguousarray(
            (inputs["W1"] * WS).reshape(NCH, 128, NCH, 4, 128)
            .transpose(2, 1, 0, 3, 4).reshape(NCH, 128, NCH, 512)).astype(E4),
        "w2": np.ascontiguousarray(w2t).astype(BF16),
        "bq": _center_heads(inputs["bq"]), "bk": _center_heads(inputs["bk"]),
        "b1": inputs["b1"].astype(np.float32), "b2": inputs["b2"].astype(np.float32),
        "fng": inputs["fn_g"].astype(np.float32),
        # fnb scaled by XS: h_sb carries the fp8 pre-scale
        "fnb": (inputs["fn_b"] * XS).astype(np.float32),
        **consts,
    }
    in_maps = []
    with_tq = with_tk = False
    for core in range(8):
        b, t0 = core // 4, (core % 4) * T
        xq_slice = np.ascontiguousarray(inputs["query"][b, t0:t0 + T].T).astype(np.float32)
        # the V projection bias is exactly additive after softmax; fold it into
        # the residual here
        xqf = xq_slice + inputs["bv"].astype(np.float32)[:, None]
        cq, sq, tq = _rope_tables(inputs["qpos"][b, t0:t0 + T],
                                  inputs["qn_g"], inputs["qn_b"])
        ck, sk, tk = _rope_tables(inputs["cpos"][b],
                                  inputs["kn_g"], inputs["kn_b"])
        m = dict(shared)
        m.update({
            "xqf": xqf, "xq": (xq_slice * XS).astype(E4),
            "xqfm": xqf.mean(axis=0, keepdims=True).astype(np.float32),
            "xc": np.ascontiguousarray(
                (inputs["context"][b].T * XS)).astype(E4),
            "cq2": cq, "sq2": sq,
            "ck2": ck, "sk2": sk,
        })
        if tq is not None:
            m["tq"] = tq
            with_tq = True
        if tk is not None:
            m["tk"] = tk
            with_tk = True
        in_maps.append(m)
    return in_maps, with_tq, with_tk


def kernel(**inputs):
    _maybe_patch_ldw_opt()
    from concourse.bass_utils import run_bass_kernel_spmd
    in_maps, with_tq, with_tk = make_in_maps(inputs)
    skip_fn = bool(np.all(np.asarray(inputs["fn_g"]) == 1.0)
                   and np.all(np.asarray(inputs["fn_b"]) == 0.0))
    skip_b2 = bool(np.all(np.asarray(inputs["b2"]) == 0.0))
    key = (with_tq, with_tk, skip_fn, skip_b2)
    if key not in _BUILT:
        _BUILT[key] = build(*key)
    nc = _BUILT[key]
    res = run_bass_kernel_spmd(nc, in_maps, core_ids=list(range(8)))
    out = np.zeros((B, N, D), np.float32)
    for core in range(8):
        b, t0 = core // 4, (core % 4) * T
        out[b, t0:t0 + T] = res.results[core]["out"].T
    return out

